# revision 19
# baseline (speedup 1.0000x reference)
"""DCT-compressed attention (nn_DCTAttentionIdeal) on 8 Trainium2 NeuronCores.

Math (per head, reference ordering):
    S    = (Q*s) @ (K*mask*s)^T with s = D**-0.25             [N,N]
    atn  = softmax(S, axis=-1)
    Vd   = Qd @ (V*mask)                                      [M,D]
    out  = Qd^T @ ((Qd @ atn @ Qd^T) @ Vd)                    [N,D]

Kernel reassociation (exact in real arithmetic):
    out = Qd^T @ (Qd @ (atn @ U)),   U = Qd^T @ (Qd @ (V*mask))  [N,D]
so the only O(N^2) contractions are the scores (N^2 D) and atn @ U
(N^2 D) -- the N^2 M path of the naive ordering is gone.

Implementation notes:
  - scores are computed TRANSPOSED (S^T[k,q]) so that exp(S^T) tiles act
    directly as matmul weights for the k-contraction of atn @ U, and the
    softmax denominator comes free as a ones-column appended to U.
  - exp is stored bf16 (fp8 mantissa is too coarse: exp quantization
    error passes straight through atn@U since Y is a random-sign sum).
  - S^T uses 2x PE row tiling (contraction D=64): even k-blocks on array
    rows 0-63, odd k-blocks on rows 64-127, chunk matmuls interleaved
    across row groups so every LDWEIGHTS hides under the other group's
    stream.
  - The kernel is paced by ScalarE's exp stream (128 activations of
    [128,1024]).  Emission is act-paced: each pair-half slot emits 4
    score matmuls + 2 activations, then pops two "filler atoms" (~8-16
    matmul slices of atn@U / DCT-tail / U-prep work for neighboring
    heads), so the strict-FIFO TensorE queue never idles on the
    act-region rotation with runnable work stuck behind it.
  - atn@U accumulators alternate between two PSUM banks so the DVE
    normalize of block q never blocks the matmuls of block q+1.
  - heads are processed in pairs for the small DCT matmuls (Vd/U/Z1/out):
    two heads' 64-wide streams share one 128-wide weight load.
  - PSUM: 6 banks = 3-deep rotation of [128,1024] score regions, 2 banks
    for the alternating atn@U accumulator / misc tail.

Sharding: batch*heads (2*16=32) split 4-per-core across 8 cores; Q_dct
replicated; no cross-core communication.  Host does layout only
(transposes / duplication for row tiling / mask broadcast layouts);
masking, softmax and all DCT algebra run on-device.
"""

from collections import deque

import numpy as np
import ml_dtypes

import concourse.tile as tile
from concourse import bacc, mybir
from concourse import bass_utils

F32 = mybir.dt.float32
BF16 = mybir.dt.bfloat16
I16 = mybir.dt.int16
ALU = mybir.AluOpType
NPBF16 = ml_dtypes.bfloat16
AF = mybir.ActivationFunctionType

B, H, N, D, M = 2, 16, 2048, 64, 256
NCORES = 8
HPC = (B * H) // NCORES  # heads per core = 4
P = 128
NT = N // P              # 16 k/q blocks
MT = M // P              # 2
SCH_A = 0.125 * 128.0 / np.log(2.0)   # Schraudolph exp: bf16 bits = A*s + B
SCH_B = 128.0 * 127.0 - 7.5


def _emit(tc, ctx, io):
    nc = tc.nc

    sh = ctx.enter_context(tc.tile_pool(name="shared", bufs=1))
    in2 = ctx.enter_context(tc.tile_pool(name="inputs", bufs=3))
    v2_pool = ctx.enter_context(tc.tile_pool(name="vpair", bufs=2))
    exp_pool = ctx.enter_context(tc.tile_pool(name="exp", bufs=2))
    ua_pool = ctx.enter_context(tc.tile_pool(name="ua", bufs=2))
    vd_pool = ctx.enter_context(tc.tile_pool(name="vd", bufs=2))
    au_pool = ctx.enter_context(tc.tile_pool(name="atnun", bufs=2))
    z1_pool = ctx.enter_context(tc.tile_pool(name="z1", bufs=2))
    ost_pool = ctx.enter_context(tc.tile_pool(name="ost", bufs=1))
    st_pool = ctx.enter_context(tc.tile_pool(name="stats", bufs=4))

    ps_st = ctx.enter_context(tc.tile_pool(name="ps_st", bufs=3, space="PSUM"))
    ps_a = ctx.enter_context(tc.tile_pool(name="ps_a", bufs=1, space="PSUM"))
    ps_m = ctx.enter_context(tc.tile_pool(name="ps_m", bufs=1, space="PSUM"))

    # --- shared, once per core (mask layouts first: first act needs them)
    mkb = sh.tile([P, NT // 2, P], BF16)  # key mask in KT2 layout
    nc.sync.dma_start(mkb[:], io["maskKB"])

    state = [None] * HPC

    def prep_dma_qk(h):
        st = state[h] = {}
        st["qt2"] = in2.tile([P, N], BF16, name="qt2", tag="qt2")
        nc.sync.dma_start(st["qt2"][:], io["QT2"][h])
        st["kt2"] = in2.tile([P, NT // 2, P], BF16, name="kt2", tag="kt2")
        nc.sync.dma_start(st["kt2"][:], io["KT2"][h])

    def prep_dma_v(h):
        st = state[h]
        if h % 2 == 0:
            st["v2"] = v2_pool.tile([P, NT, 2, D], BF16, name="v2", tag="v2")
        else:
            st["v2"] = state[h - 1]["v2"]
        nc.sync.dma_start(
            st["v2"][:, :, h % 2, :], io["V"][h].rearrange("(t p) d -> p t d", p=P)
        )

    def prep_dma(h):
        prep_dma_qk(h)
        prep_dma_v(h)

    prep_dma_qk(0)
    prep_dma_v(0)
    mvb = sh.tile([P, NT, D], BF16)       # value mask in V layout
    nc.sync.dma_start(mvb[:], io["maskVB"])

    qdtr = sh.tile([P, NT, M], BF16)    # Qd^T[n, m] -> [p, t, m]
    nc.sync.dma_start(qdtr[:], io["QdTr"].rearrange("(t p) m -> p t m", p=P))
    qdnr = sh.tile([P, MT, N], BF16)    # Qd[m, q]   -> [p, c, q]
    nc.sync.dma_start(qdnr[:], io["QdNr"].rearrange("(c p) q -> p c q", p=P))

    if HPC > 1:
        prep_dma(1)

    # dummy activation: pull the exp table load off the critical path
    scr = sh.tile([P, 1], F32)
    nc.vector.memset(scr[:], 0.0)
    nc.scalar.activation(scr[:], scr[:], AF.Exp)

    mkbf = mkb.rearrange("p a b -> p (a b)")

    def dummy_mms(n):
        """Keep the PE's HAM activity monitor busy (K=8/8) when the pipeline
        has no real TensorE work: harmless matmuls over the mask tile."""
        psd = ps_m.tile([P, 512], F32, name="psd", tag="misc")
        for _ in range(n):
            nc.tensor.matmul(
                psd[:], lhsT=mkbf[:, 0:P], rhs=mkbf[:, 0:512], start=True, stop=True
            )

    # warm-up burst: ~7us of back-to-back matmuls during the DMA wait flips
    # the HAM clock gate to full speed before the real pipeline starts.
    dummy_mms(16)

    def mask_k(h):
        st = state[h]
        nc.vector.tensor_mul(st["kt2"][:], st["kt2"][:], mkb[:])

    def mask_v(h):
        st = state[h]
        nc.vector.tensor_mul(
            st["v2"][:, :, h % 2, :], st["v2"][:, :, h % 2, :], mvb[:]
        )

    # ---- filler atoms (TensorE work hidden inside the act-paced spine) --
    # Small DCT matmuls run head-PAIRED: rhs [128, 2*64] spans both heads,
    # so one 128-column weight load feeds 128 streamed columns.
    def vd_atom(hp):
        """Vd = Qd @ (V*m) for head pair hp -> vd2 [128, 2, 2, 64]."""
        st = state[2 * hp]
        vd2 = st["vd2"] = vd_pool.tile([P, MT, 2, D], BF16, name="vd2", tag="vd2")
        psv = ps_m.tile([P, 512], F32, name="psv", tag="misc")
        for mh in range(MT):
            for t in range(NT):
                nc.tensor.matmul(
                    psv[:, mh * 2 * D : (mh + 1) * 2 * D],
                    lhsT=qdtr[:, t, mh * P : (mh + 1) * P],
                    rhs=st["v2"][:, t, :, :],
                    start=(t == 0),
                    stop=(t == NT - 1),
                )
        nc.vector.tensor_copy(
            vd2[:], psv[:, 0 : MT * 2 * D].rearrange("p (c h d) -> p c h d", h=2, d=D)
        )

    def u_atom(hp, quarter):
        """U = Qd^T @ Vd for 4 k-blocks of head pair hp (+ ones column)."""
        st = state[2 * hp]
        if quarter == 0:
            st["ua2"] = ua_pool.tile([P, NT, 2, D + 1], BF16, name="ua2", tag="ua2")
        ua2 = st["ua2"]
        psu = ps_m.tile([P, 512], F32, name="psu", tag="misc")
        for k4 in range(4):
            kc = quarter * 4 + k4
            for mh in range(MT):
                nc.tensor.matmul(
                    psu[:, k4 * 2 * D : (k4 + 1) * 2 * D],
                    lhsT=qdnr[:, mh, kc * P : (kc + 1) * P],
                    rhs=st["vd2"][:, mh, :, :],
                    start=(mh == 0),
                    stop=(mh == MT - 1),
                )
        nc.vector.tensor_copy(
            ua2[:, quarter * 4 : (quarter + 1) * 4, :, 0:D],
            psu[:].rearrange("p (c h d) -> p c h d", h=2, d=D),
        )
        nc.vector.memset(ua2[:, quarter * 4 : (quarter + 1) * 4, :, D : D + 1], 1.0)

    def atnu_atom(h, q):
        """One q-block of Y = atn @ [U|1]: 16 matmuls + normalize.
        Accumulator bank alternates between ps_a and ps_m so the DVE
        normalize of block q never blocks the matmuls of block q+1."""
        st = state[h]
        hp2 = h % 2
        if q == 0 and hp2 == 0:
            state[h]["au2"] = au_pool.tile([P, NT, 2, D], BF16, name="au2", tag="au2")
        if q == 0 and hp2 == 1:
            state[h]["au2"] = state[h - 1]["au2"]
        au2 = st["au2"]
        ua2 = state[h - hp2]["ua2"]
        pool = ps_a if q % 2 == 0 else ps_m
        tag = "psa" if q % 2 == 0 else "misc"
        psa = pool.tile([P, D + 1], F32, name="psa", tag=tag)
        for kc in range(NT):
            nc.tensor.matmul(
                psa[:],
                lhsT=st["ex"][:, kc, q * P : (q + 1) * P],
                rhs=ua2[:, kc, hp2, :],
                start=(kc == 0),
                stop=(kc == NT - 1),
            )
        rec = st_pool.tile([P, 1], F32, name="rec", tag="rec")
        nc.vector.reciprocal(rec[:], psa[:, D : D + 1])
        if h == HPC - 1:
            nc.scalar.mul(au2[:, q, hp2, :], psa[:, 0:D], rec[:])
        else:
            nc.vector.tensor_scalar_mul(au2[:, q, hp2, :], psa[:, 0:D], rec[:])

    def z1_atom(hp):
        """Z1 = Qd @ Y for head pair hp -> z12 [128, 2, 2, 64]."""
        st = state[2 * hp]
        z12 = st["z12"] = z1_pool.tile([P, MT, 2, D], BF16, name="z12", tag="z12")
        psz = ps_m.tile([P, 512], F32, name="psz", tag="misc")
        for mh in range(MT):
            for q in range(NT):
                nc.tensor.matmul(
                    psz[:, mh * 2 * D : (mh + 1) * 2 * D],
                    lhsT=qdtr[:, q, mh * P : (mh + 1) * P],
                    rhs=st["au2"][:, q, :, :],
                    start=(q == 0),
                    stop=(q == NT - 1),
                )
        nc.vector.tensor_copy(
            z12[:], psz[:, 0 : MT * 2 * D].rearrange("p (c h d) -> p c h d", h=2, d=D)
        )

    def out_atom(hp, quarter):
        """out = Qd^T @ Z1 for 4 q-blocks of head pair hp; DMA at the end."""
        st = state[2 * hp]
        if quarter == 0:
            st["ost2"] = ost_pool.tile([P, NT, 2, D], F32, name="ost2", tag="ost2")
        ost2 = st["ost2"]
        pso = ps_m.tile([P, 512], F32, name="pso", tag="misc")
        for q4 in range(4):
            q = quarter * 4 + q4
            for mh in range(MT):
                nc.tensor.matmul(
                    pso[:, q4 * 2 * D : (q4 + 1) * 2 * D],
                    lhsT=qdnr[:, mh, q * P : (q + 1) * P],
                    rhs=st["z12"][:, mh, :, :],
                    start=(mh == 0),
                    stop=(mh == MT - 1),
                )
        nc.vector.tensor_copy(
            ost2[:, quarter * 4 : (quarter + 1) * 4, :, :],
            pso[:].rearrange("p (c h d) -> p c h d", h=2, d=D),
        )
        if quarter % 2 == 1:
            for hh in range(2):
                nc.sync.dma_start(
                    io["out"][2 * hp + hh]
                    .rearrange("(t p) d -> p t d", p=P)[:, (quarter - 1) * 4 : (quarter + 1) * 4, :],
                    ost2[:, (quarter - 1) * 4 : (quarter + 1) * 4, hh, :],
                )
        if quarter == 3:
            state[2 * hp] = state[2 * hp + 1] = None

    def phase_atoms(h):
        """Fillers for the S-phase of head h (indices may refer to earlier
        heads' work whose inputs completed in previous phases).  Dummy
        matmuls pad phases whose real TensorE work is far below the act
        spine, so the HAM clock gate never re-throttles the PE."""
        atoms = []
        prep = []
        if h % 2 == 1 and h < HPC:
            hp = h // 2  # prep for the pair whose 2nd head phase this is
            prep.append(lambda hp=hp: vd_atom(hp))
            for qu in range(4):
                prep.append(lambda hp=hp, qu=qu: u_atom(hp, qu))
        work = []
        if 0 <= h - 1 < HPC:
            for q in range(NT):
                work.append(lambda h=h, q=q: atnu_atom(h - 1, q))
        tail = []
        if h - 2 >= 1 and (h - 2) % 2 == 1:
            hp = (h - 2) // 2  # tail for the pair completed at end of h-1
            tail.append(lambda hp=hp: z1_atom(hp))
            for qu in range(4):
                tail.append(lambda hp=hp, qu=qu: out_atom(hp, qu))
        n_dum = {0: 10, 1: 2, 2: 2, 3: 2}.get(h, 0)
        atoms = prep + work + tail
        # interleave dummies after every 2nd real atom
        if n_dum:
            out = []
            k = 0
            for a in atoms:
                out.append(a)
                k += 1
                if k % 2 == 0 and n_dum > 0:
                    out.append(lambda: dummy_mms(4))
                    n_dum -= 1
            out.extend([lambda: dummy_mms(4)] * n_dum)
            atoms = out
        return atoms

    def s_pair_half(h, j, half):
        """Score k-blocks (2j, 2j+1) for one q-half: 4 matmuls interleaved
        across PE row groups + 2 exp evaluations.  In phases 0 and HPC-1
        (which lack filler work / gate the epilogue) some units compute exp
        on the DVE instead via the Schraudolph bit trick: bf16 bits =
        round(A*scores + B), evaluated as one tensor_scalar into an int16
        view of the exp tile (rel err ~1.8%, cancels partly in softmax)."""
        st = state[h]
        slot = 2 * j + half
        if h in (0, HPC - 1):
            sch = 2 <= slot <= 13
        else:
            sch = 8 <= slot <= 13
        pss = []
        for side in range(2):
            pss.append(ps_st.tile([P, 1024], F32, name="st", tag="st"))
        for c in range(2):
            for side in range(2):
                pr = slice(64 * side, 64 * (side + 1))
                q0 = half * 1024 + c * 512
                nc.tensor.matmul(
                    pss[side][:, c * 512 : (c + 1) * 512],
                    lhsT=st["kt2"][pr, j, :],
                    rhs=st["qt2"][pr, q0 : q0 + 512],
                    start=True,
                    stop=True,
                )
        for side in range(2):
            kc = 2 * j + side
            dst = st["ex"][:, kc, half * 1024 : (half + 1) * 1024]
            if sch and side == 1:
                nc.vector.tensor_scalar(
                    dst.bitcast(I16), pss[side][:], SCH_A, SCH_B,
                    op0=ALU.mult, op1=ALU.add,
                )
            else:
                nc.scalar.activation(dst, pss[side][:], AF.Exp, scale=0.125)

    # --- act-paced pipelined emission -----------------------------------
    mask_k(0)
    atoms = deque()
    for h in range(HPC):
        st = state[h]
        st["ex"] = exp_pool.tile([P, NT, N], BF16, name="ex", tag="ex")
        atoms.extend(phase_atoms(h))
        for slot in range(NT):
            j, half = divmod(slot, 2)
            if slot == 2 and h == 0:
                mask_v(0)
            if slot == 4 and h + 1 < HPC:
                mask_k(h + 1)
                mask_v(h + 1)
            if slot == 8 and h + 2 < HPC:
                prep_dma(h + 2)
            s_pair_half(h, j, half)
            if h > 0 or slot >= 2:
                want = (len(atoms) + (NT - 1 - slot)) // (NT - slot)
                for _ in range(min(want, 3)):
                    if atoms:
                        atoms.popleft()()
    for h in (HPC, HPC + 1):
        atoms.extend(phase_atoms(h))
    while atoms:
        atoms.popleft()()


def build_nc():
    from contextlib import ExitStack

    nc = bacc.Bacc("TRN2", target_bir_lowering=False, debug=False)
    io = {
        "QT2": nc.dram_tensor("QT2", [HPC, P, N], BF16, kind="ExternalInput").ap(),
        "KT2": nc.dram_tensor("KT2", [HPC, P, NT // 2, P], BF16, kind="ExternalInput").ap(),
        "V": nc.dram_tensor("V", [HPC, N, D], BF16, kind="ExternalInput").ap(),
        "maskKB": nc.dram_tensor("maskKB", [P, NT // 2, P], BF16, kind="ExternalInput").ap(),
        "maskVB": nc.dram_tensor("maskVB", [P, NT, D], BF16, kind="ExternalInput").ap(),
        "QdTr": nc.dram_tensor("QdTr", [N, M], BF16, kind="ExternalInput").ap(),
        "QdNr": nc.dram_tensor("QdNr", [M, N], BF16, kind="ExternalInput").ap(),
        "out": nc.dram_tensor("out", [HPC, N, D], F32, kind="ExternalOutput").ap(),
    }
    with tile.TileContext(nc) as tc:
        with ExitStack() as ctx:
            _emit(tc, ctx, io)
    nc.compile()
    return nc


_NC = None


def _get_nc():
    global _NC
    if _NC is None:
        _NC = build_nc()
    return _NC


def make_in_maps(Q, K, V, mask, Q_dct):
    Q = np.asarray(Q, dtype=np.float32).reshape(B * H, N, D)
    K = np.asarray(K, dtype=np.float32).reshape(B * H, N, D)
    V = np.asarray(V, dtype=np.float32).reshape(B * H, N, D)
    mask = np.asarray(mask, dtype=np.float32)
    Q_dct = np.asarray(Q_dct, dtype=np.float32)

    QT = np.ascontiguousarray(Q.transpose(0, 2, 1))          # [BH, 64, N]
    KT = np.ascontiguousarray(K.transpose(0, 2, 1))          # [BH, 64, N]
    # duplicate Q^T across both partition halves (PE row tiling)
    QT2 = np.concatenate([QT, QT], axis=1)                   # [BH, 128, N]
    # interleave K^T k-blocks: even blocks on partitions 0-63, odd on 64-127
    KTb = KT.reshape(B * H, D, NT, P)                        # [BH, 64, 16, 128]
    KT2 = np.concatenate([KTb[:, :, 0::2, :], KTb[:, :, 1::2, :]], axis=1)
    QdTr = np.ascontiguousarray(Q_dct.T).astype(NPBF16)
    QdNr = np.ascontiguousarray(Q_dct).astype(NPBF16)

    in_maps = []
    for c in range(NCORES):
        sl = slice(HPC * c, HPC * (c + 1))
        heads = list(range(HPC * c, HPC * (c + 1)))
        bs = {hp // H for hp in heads}
        assert len(bs) == 1, "all heads on a core must share a batch row"
        b = bs.pop()
        # key mask in KT2 layout [128, 8, 128] (broadcast over d-partitions)
        mk = mask[b].reshape(NT, P)                          # [16 blocks, 128]
        mkb = np.empty((P, NT // 2, P), dtype=np.float32)
        mkb[0:64] = mk[0::2][None, :, :]
        mkb[64:128] = mk[1::2][None, :, :]
        # value mask in V-tile layout [128, 16, 64]
        mvb = np.broadcast_to(mk.T[:, :, None], (P, NT, D))
        in_maps.append(
            {
                "QT2": np.ascontiguousarray(QT2[sl]).astype(NPBF16),
                "KT2": np.ascontiguousarray(KT2[sl]).astype(NPBF16),
                "V": np.ascontiguousarray(V[sl]).astype(NPBF16),
                "maskKB": np.ascontiguousarray(mkb).astype(NPBF16),
                "maskVB": np.ascontiguousarray(mvb).astype(NPBF16),
                "QdTr": QdTr,
                "QdNr": QdNr,
            }
        )
    return in_maps


def run_on_device(in_maps, **kwargs):
    nc = _get_nc()
    return bass_utils.run_bass_kernel_spmd(
        nc, in_maps, core_ids=list(range(NCORES)), **kwargs
    )


def kernel(Q, K, V, mask, Q_dct):
    in_maps = make_in_maps(Q, K, V, mask, Q_dct)
    res = run_on_device(in_maps)
    out = np.empty((B * H, N, D), dtype=np.float32)
    for c in range(NCORES):
        out[HPC * c : HPC * (c + 1)] = res.results[c]["out"]
    return out.reshape(B, H, N, D)


# revision 20
# speedup vs baseline: 1.2171x; 1.2171x over previous
"""DCT-compressed attention (nn_DCTAttentionIdeal) on 8 Trainium2 NeuronCores.

Math (per head, reference ordering):
    S    = (Q*s) @ (K*mask*s)^T with s = D**-0.25             [N,N]
    atn  = softmax(S, axis=-1)
    Vd   = Qd @ (V*mask)                                      [M,D]
    out  = Qd^T @ ((Qd @ atn @ Qd^T) @ Vd)                    [N,D]

Kernel reassociation (exact in real arithmetic):
    out = Qd^T @ (Qd @ (atn @ U)),   U = Qd^T @ (Qd @ (V*mask))  [N,D]
so the only O(N^2) contractions are the scores (N^2 D) and atn @ U
(N^2 D) -- the N^2 M path of the naive ordering is gone.

Implementation notes:
  - scores are computed TRANSPOSED (S^T[k,q]) so that exp(S^T) tiles act
    directly as matmul weights for the k-contraction of atn @ U, and the
    softmax denominator comes free as a ones-column appended to U.
  - exp is stored bf16 (fp8 mantissa is too coarse: exp quantization
    error passes straight through atn@U since Y is a random-sign sum).
  - S^T uses 2x PE row tiling (contraction D=64): even k-blocks on array
    rows 0-63, odd k-blocks on rows 64-127, chunk matmuls interleaved
    across row groups so every LDWEIGHTS hides under the other group's
    stream.
  - The kernel is paced by ScalarE's exp stream (128 activations of
    [128,1024]).  Emission is act-paced: each pair-half slot emits 4
    score matmuls + 2 activations, then pops two "filler atoms" (~8-16
    matmul slices of atn@U / DCT-tail / U-prep work for neighboring
    heads), so the strict-FIFO TensorE queue never idles on the
    act-region rotation with runnable work stuck behind it.
  - atn@U accumulators alternate between two PSUM banks so the DVE
    normalize of block q never blocks the matmuls of block q+1.
  - heads are processed in pairs for the small DCT matmuls (Vd/U/Z1/out):
    two heads' 64-wide streams share one 128-wide weight load.
  - PSUM: 6 banks = 3-deep rotation of [128,1024] score regions, 2 banks
    for the alternating atn@U accumulator / misc tail.

Sharding: batch*heads (2*16=32) split 4-per-core across 8 cores; Q_dct
replicated; no cross-core communication.  Host does layout only
(transposes / duplication for row tiling / mask broadcast layouts);
masking, softmax and all DCT algebra run on-device.
"""

from collections import deque

import numpy as np
import ml_dtypes

import concourse.tile as tile
from concourse import bacc, mybir
from concourse import bass_utils

F32 = mybir.dt.float32
BF16 = mybir.dt.bfloat16
I16 = mybir.dt.int16
ALU = mybir.AluOpType
NPBF16 = ml_dtypes.bfloat16
AF = mybir.ActivationFunctionType

B, H, N, D, M = 2, 16, 2048, 64, 256
NCORES = 8
HPC = (B * H) // NCORES  # heads per core = 4
P = 128
NT = N // P              # 16 k/q blocks
MT = M // P              # 2
SCH_A = 0.125 * 128.0 / np.log(2.0)   # Schraudolph exp: bf16 bits = A*s + B
SCH_B = 128.0 * 127.0 - 7.5


def _emit(tc, ctx, io):
    nc = tc.nc

    sh = ctx.enter_context(tc.tile_pool(name="shared", bufs=1))
    in2 = ctx.enter_context(tc.tile_pool(name="inputs", bufs=3))
    v2_pool = ctx.enter_context(tc.tile_pool(name="vpair", bufs=2))
    exp_pool = ctx.enter_context(tc.tile_pool(name="exp", bufs=2))
    ua_pool = ctx.enter_context(tc.tile_pool(name="ua", bufs=2))
    vd_pool = ctx.enter_context(tc.tile_pool(name="vd", bufs=2))
    au_pool = ctx.enter_context(tc.tile_pool(name="atnun", bufs=2))
    z1_pool = ctx.enter_context(tc.tile_pool(name="z1", bufs=2))
    ost_pool = ctx.enter_context(tc.tile_pool(name="ost", bufs=1))
    st_pool = ctx.enter_context(tc.tile_pool(name="stats", bufs=4))

    ps_st = ctx.enter_context(tc.tile_pool(name="ps_st", bufs=3, space="PSUM"))
    ps_a = ctx.enter_context(tc.tile_pool(name="ps_a", bufs=1, space="PSUM"))
    ps_m = ctx.enter_context(tc.tile_pool(name="ps_m", bufs=1, space="PSUM"))

    # --- shared, once per core (mask layouts first: first act needs them)
    mkb = sh.tile([P, NT // 2, P], BF16)  # key mask in KT2 layout
    nc.sync.dma_start(mkb[:], io["maskKB"])

    state = [None] * HPC

    def prep_dma(h):
        st = state[h] = {}
        st["qt2"] = in2.tile([P, N], BF16, name="qt2", tag="qt2")
        nc.sync.dma_start(st["qt2"][:], io["QT2"][h])
        st["kt2"] = in2.tile([P, NT // 2, P], BF16, name="kt2", tag="kt2")
        nc.sync.dma_start(st["kt2"][:], io["KT2"][h])
        if h % 2 == 0:
            st["v2"] = v2_pool.tile([P, NT, 2, D], BF16, name="v2", tag="v2")
        else:
            st["v2"] = state[h - 1]["v2"]
        nc.sync.dma_start(
            st["v2"][:, :, h % 2, :], io["V"][h].rearrange("(t p) d -> p t d", p=P)
        )

    prep_dma(0)
    mvb = sh.tile([P, NT, D], BF16)       # value mask in V layout
    nc.sync.dma_start(mvb[:], io["maskVB"])

    qdtr = sh.tile([P, NT, M], BF16)    # Qd^T[n, m] -> [p, t, m]
    nc.sync.dma_start(qdtr[:], io["QdTr"].rearrange("(t p) m -> p t m", p=P))
    qdnr = sh.tile([P, MT, N], BF16)    # Qd[m, q]   -> [p, c, q]
    nc.sync.dma_start(qdnr[:], io["QdNr"].rearrange("(c p) q -> p c q", p=P))

    if HPC > 1:
        prep_dma(1)

    # dummy activation: pull the exp table load off the critical path
    scr = sh.tile([P, 1], F32)
    nc.vector.memset(scr[:], 0.0)
    nc.scalar.activation(scr[:], scr[:], AF.Exp)

    mkbf = mkb.rearrange("p a b -> p (a b)")

    def dummy_mms(n):
        """Keep the PE's HAM activity monitor busy (K=8/8) when the pipeline
        has no real TensorE work: harmless matmuls over the mask tile."""
        psd = ps_m.tile([P, 512], F32, name="psd", tag="misc")
        for _ in range(n):
            nc.tensor.matmul(
                psd[:], lhsT=mkbf[:, 0:P], rhs=mkbf[:, 0:512], start=True, stop=True
            )

    # warm-up burst: ~7us of back-to-back matmuls during the DMA wait flips
    # the HAM clock gate to full speed before the real pipeline starts.
    dummy_mms(16)

    def mask_inputs(h):
        st = state[h]
        nc.vector.tensor_mul(st["kt2"][:], st["kt2"][:], mkb[:])
        nc.vector.tensor_mul(
            st["v2"][:, :, h % 2, :], st["v2"][:, :, h % 2, :], mvb[:]
        )

    # ---- filler atoms (TensorE work hidden inside the act-paced spine) --
    # Small DCT matmuls run head-PAIRED: rhs [128, 2*64] spans both heads,
    # so one 128-column weight load feeds 128 streamed columns.
    def vd_atom(hp):
        """Vd = Qd @ (V*m) for head pair hp -> vd2 [128, 2, 2, 64]."""
        st = state[2 * hp]
        vd2 = st["vd2"] = vd_pool.tile([P, MT, 2, D], BF16, name="vd2", tag="vd2")
        psv = ps_m.tile([P, 512], F32, name="psv", tag="misc")
        for mh in range(MT):
            for t in range(NT):
                nc.tensor.matmul(
                    psv[:, mh * 2 * D : (mh + 1) * 2 * D],
                    lhsT=qdtr[:, t, mh * P : (mh + 1) * P],
                    rhs=st["v2"][:, t, :, :],
                    start=(t == 0),
                    stop=(t == NT - 1),
                )
        nc.vector.tensor_copy(
            vd2[:], psv[:, 0 : MT * 2 * D].rearrange("p (c h d) -> p c h d", h=2, d=D)
        )

    def u_atom(hp, quarter):
        """U = Qd^T @ Vd for 4 k-blocks of head pair hp (+ ones column)."""
        st = state[2 * hp]
        if quarter == 0:
            st["ua2"] = ua_pool.tile([P, NT, 2, D + 1], BF16, name="ua2", tag="ua2")
        ua2 = st["ua2"]
        psu = ps_m.tile([P, 512], F32, name="psu", tag="misc")
        for k4 in range(4):
            kc = quarter * 4 + k4
            for mh in range(MT):
                nc.tensor.matmul(
                    psu[:, k4 * 2 * D : (k4 + 1) * 2 * D],
                    lhsT=qdnr[:, mh, kc * P : (kc + 1) * P],
                    rhs=st["vd2"][:, mh, :, :],
                    start=(mh == 0),
                    stop=(mh == MT - 1),
                )
        nc.vector.tensor_copy(
            ua2[:, quarter * 4 : (quarter + 1) * 4, :, 0:D],
            psu[:].rearrange("p (c h d) -> p c h d", h=2, d=D),
        )
        nc.vector.memset(ua2[:, quarter * 4 : (quarter + 1) * 4, :, D : D + 1], 1.0)

    def atnu_atom(h, q):
        """One q-block of Y = atn @ [U|1]: 16 matmuls + normalize.
        Accumulator bank alternates between ps_a and ps_m so the DVE
        normalize of block q never blocks the matmuls of block q+1."""
        st = state[h]
        hp2 = h % 2
        if q == 0 and hp2 == 0:
            state[h]["au2"] = au_pool.tile([P, NT, 2, D], BF16, name="au2", tag="au2")
        if q == 0 and hp2 == 1:
            state[h]["au2"] = state[h - 1]["au2"]
        au2 = st["au2"]
        ua2 = state[h - hp2]["ua2"]
        pool = ps_a if q % 2 == 0 else ps_m
        tag = "psa" if q % 2 == 0 else "misc"
        psa = pool.tile([P, D + 1], F32, name="psa", tag=tag)
        for kc in range(NT):
            nc.tensor.matmul(
                psa[:],
                lhsT=st["ex"][:, kc, q * P : (q + 1) * P],
                rhs=ua2[:, kc, hp2, :],
                start=(kc == 0),
                stop=(kc == NT - 1),
            )
        rec = st_pool.tile([P, 1], F32, name="rec", tag="rec")
        nc.vector.reciprocal(rec[:], psa[:, D : D + 1])
        if h == HPC - 1:
            nc.scalar.mul(au2[:, q, hp2, :], psa[:, 0:D], rec[:])
        else:
            nc.vector.tensor_scalar_mul(au2[:, q, hp2, :], psa[:, 0:D], rec[:])

    def z1_atom(hp):
        """Z1 = Qd @ Y for head pair hp -> z12 [128, 2, 2, 64]."""
        st = state[2 * hp]
        z12 = st["z12"] = z1_pool.tile([P, MT, 2, D], BF16, name="z12", tag="z12")
        psz = ps_m.tile([P, 512], F32, name="psz", tag="misc")
        for mh in range(MT):
            for q in range(NT):
                nc.tensor.matmul(
                    psz[:, mh * 2 * D : (mh + 1) * 2 * D],
                    lhsT=qdtr[:, q, mh * P : (mh + 1) * P],
                    rhs=st["au2"][:, q, :, :],
                    start=(q == 0),
                    stop=(q == NT - 1),
                )
        nc.vector.tensor_copy(
            z12[:], psz[:, 0 : MT * 2 * D].rearrange("p (c h d) -> p c h d", h=2, d=D)
        )

    def out_atom(hp, quarter):
        """out = Qd^T @ Z1 for 4 q-blocks of head pair hp; DMA at the end."""
        st = state[2 * hp]
        if quarter == 0:
            st["ost2"] = ost_pool.tile([P, NT, 2, D], F32, name="ost2", tag="ost2")
        ost2 = st["ost2"]
        pso = ps_m.tile([P, 512], F32, name="pso", tag="misc")
        for q4 in range(4):
            q = quarter * 4 + q4
            for mh in range(MT):
                nc.tensor.matmul(
                    pso[:, q4 * 2 * D : (q4 + 1) * 2 * D],
                    lhsT=qdnr[:, mh, q * P : (q + 1) * P],
                    rhs=st["z12"][:, mh, :, :],
                    start=(mh == 0),
                    stop=(mh == MT - 1),
                )
        nc.vector.tensor_copy(
            ost2[:, quarter * 4 : (quarter + 1) * 4, :, :],
            pso[:].rearrange("p (c h d) -> p c h d", h=2, d=D),
        )
        if quarter % 2 == 1:
            for hh in range(2):
                nc.sync.dma_start(
                    io["out"][2 * hp + hh]
                    .rearrange("(t p) d -> p t d", p=P)[:, (quarter - 1) * 4 : (quarter + 1) * 4, :],
                    ost2[:, (quarter - 1) * 4 : (quarter + 1) * 4, hh, :],
                )
        if quarter == 3:
            state[2 * hp] = state[2 * hp + 1] = None

    def phase_atoms(h):
        """Fillers for the S-phase of head h (indices may refer to earlier
        heads' work whose inputs completed in previous phases).  Dummy
        matmuls pad phases whose real TensorE work is far below the act
        spine, so the HAM clock gate never re-throttles the PE."""
        atoms = []
        prep = []
        if h % 2 == 1 and h < HPC:
            hp = h // 2  # prep for the pair whose 2nd head phase this is
            prep.append(lambda hp=hp: vd_atom(hp))
            for qu in range(4):
                prep.append(lambda hp=hp, qu=qu: u_atom(hp, qu))
        work = []
        if 0 <= h - 1 < HPC:
            for q in range(NT):
                work.append(lambda h=h, q=q: atnu_atom(h - 1, q))
        tail = []
        if h - 2 >= 1 and (h - 2) % 2 == 1:
            hp = (h - 2) // 2  # tail for the pair completed at end of h-1
            tail.append(lambda hp=hp: z1_atom(hp))
            for qu in range(4):
                tail.append(lambda hp=hp, qu=qu: out_atom(hp, qu))
        n_dum = {0: 10, 1: 2, 2: 2, 3: 2}.get(h, 0)
        atoms = prep + work + tail
        # interleave dummies after every 2nd real atom
        if n_dum:
            out = []
            k = 0
            for a in atoms:
                out.append(a)
                k += 1
                if k % 2 == 0 and n_dum > 0:
                    out.append(lambda: dummy_mms(4))
                    n_dum -= 1
            out.extend([lambda: dummy_mms(4)] * n_dum)
            atoms = out
        return atoms

    def s_pair_half(h, j, half):
        """Score k-blocks (2j, 2j+1) for one q-half: 4 matmuls interleaved
        across PE row groups + 2 exp evaluations.  In phases 0 and HPC-1
        (which lack filler work / gate the epilogue) some units compute exp
        on the DVE instead via the Schraudolph bit trick: bf16 bits =
        round(A*scores + B), evaluated as one tensor_scalar into an int16
        view of the exp tile (rel err ~1.8%, cancels partly in softmax)."""
        st = state[h]
        slot = 2 * j + half
        if h in (0, HPC - 1):
            sch = 2 <= slot <= 13
        else:
            sch = 8 <= slot <= 13
        pss = []
        for side in range(2):
            pss.append(ps_st.tile([P, 1024], F32, name="st", tag="st"))
        for c in range(2):
            for side in range(2):
                pr = slice(64 * side, 64 * (side + 1))
                q0 = half * 1024 + c * 512
                nc.tensor.matmul(
                    pss[side][:, c * 512 : (c + 1) * 512],
                    lhsT=st["kt2"][pr, j, :],
                    rhs=st["qt2"][pr, q0 : q0 + 512],
                    start=True,
                    stop=True,
                )
        for side in range(2):
            kc = 2 * j + side
            dst = st["ex"][:, kc, half * 1024 : (half + 1) * 1024]
            if sch and side == 1:
                nc.vector.tensor_scalar(
                    dst.bitcast(I16), pss[side][:], SCH_A, SCH_B,
                    op0=ALU.mult, op1=ALU.add,
                )
            else:
                nc.scalar.activation(dst, pss[side][:], AF.Exp, scale=0.125)

    # --- act-paced pipelined emission -----------------------------------
    mask_inputs(0)
    atoms = deque()
    for h in range(HPC):
        st = state[h]
        st["ex"] = exp_pool.tile([P, NT, N], BF16, name="ex", tag="ex")
        atoms.extend(phase_atoms(h))
        for slot in range(NT):
            j, half = divmod(slot, 2)
            if slot == 4 and h + 1 < HPC:
                mask_inputs(h + 1)
            if slot == 8 and h + 2 < HPC:
                prep_dma(h + 2)
            s_pair_half(h, j, half)
            if h > 0 or slot >= 2:
                want = (len(atoms) + (NT - 1 - slot)) // (NT - slot)
                for _ in range(min(want, 3)):
                    if atoms:
                        atoms.popleft()()
    for h in (HPC, HPC + 1):
        atoms.extend(phase_atoms(h))
    while atoms:
        atoms.popleft()()


def build_nc():
    from contextlib import ExitStack

    nc = bacc.Bacc("TRN2", target_bir_lowering=False, debug=False)
    io = {
        "QT2": nc.dram_tensor("QT2", [HPC, P, N], BF16, kind="ExternalInput").ap(),
        "KT2": nc.dram_tensor("KT2", [HPC, P, NT // 2, P], BF16, kind="ExternalInput").ap(),
        "V": nc.dram_tensor("V", [HPC, N, D], BF16, kind="ExternalInput").ap(),
        "maskKB": nc.dram_tensor("maskKB", [P, NT // 2, P], BF16, kind="ExternalInput").ap(),
        "maskVB": nc.dram_tensor("maskVB", [P, NT, D], BF16, kind="ExternalInput").ap(),
        "QdTr": nc.dram_tensor("QdTr", [N, M], BF16, kind="ExternalInput").ap(),
        "QdNr": nc.dram_tensor("QdNr", [M, N], BF16, kind="ExternalInput").ap(),
        "out": nc.dram_tensor("out", [HPC, N, D], F32, kind="ExternalOutput").ap(),
    }
    with tile.TileContext(nc) as tc:
        with ExitStack() as ctx:
            _emit(tc, ctx, io)
    nc.compile()
    return nc


_NC = None


def _get_nc():
    global _NC
    if _NC is None:
        _NC = build_nc()
    return _NC


def make_in_maps(Q, K, V, mask, Q_dct):
    Q = np.asarray(Q, dtype=np.float32).reshape(B * H, N, D)
    K = np.asarray(K, dtype=np.float32).reshape(B * H, N, D)
    V = np.asarray(V, dtype=np.float32).reshape(B * H, N, D)
    mask = np.asarray(mask, dtype=np.float32)
    Q_dct = np.asarray(Q_dct, dtype=np.float32)

    QT = np.ascontiguousarray(Q.transpose(0, 2, 1))          # [BH, 64, N]
    KT = np.ascontiguousarray(K.transpose(0, 2, 1))          # [BH, 64, N]
    # duplicate Q^T across both partition halves (PE row tiling)
    QT2 = np.concatenate([QT, QT], axis=1)                   # [BH, 128, N]
    # interleave K^T k-blocks: even blocks on partitions 0-63, odd on 64-127
    KTb = KT.reshape(B * H, D, NT, P)                        # [BH, 64, 16, 128]
    KT2 = np.concatenate([KTb[:, :, 0::2, :], KTb[:, :, 1::2, :]], axis=1)
    QdTr = np.ascontiguousarray(Q_dct.T).astype(NPBF16)
    QdNr = np.ascontiguousarray(Q_dct).astype(NPBF16)

    in_maps = []
    for c in range(NCORES):
        sl = slice(HPC * c, HPC * (c + 1))
        heads = list(range(HPC * c, HPC * (c + 1)))
        bs = {hp // H for hp in heads}
        assert len(bs) == 1, "all heads on a core must share a batch row"
        b = bs.pop()
        # key mask in KT2 layout [128, 8, 128] (broadcast over d-partitions)
        mk = mask[b].reshape(NT, P)                          # [16 blocks, 128]
        mkb = np.empty((P, NT // 2, P), dtype=np.float32)
        mkb[0:64] = mk[0::2][None, :, :]
        mkb[64:128] = mk[1::2][None, :, :]
        # value mask in V-tile layout [128, 16, 64]
        mvb = np.broadcast_to(mk.T[:, :, None], (P, NT, D))
        in_maps.append(
            {
                "QT2": np.ascontiguousarray(QT2[sl]).astype(NPBF16),
                "KT2": np.ascontiguousarray(KT2[sl]).astype(NPBF16),
                "V": np.ascontiguousarray(V[sl]).astype(NPBF16),
                "maskKB": np.ascontiguousarray(mkb).astype(NPBF16),
                "maskVB": np.ascontiguousarray(mvb).astype(NPBF16),
                "QdTr": QdTr,
                "QdNr": QdNr,
            }
        )
    return in_maps


def run_on_device(in_maps, **kwargs):
    nc = _get_nc()
    return bass_utils.run_bass_kernel_spmd(
        nc, in_maps, core_ids=list(range(NCORES)), **kwargs
    )


def kernel(Q, K, V, mask, Q_dct):
    in_maps = make_in_maps(Q, K, V, mask, Q_dct)
    res = run_on_device(in_maps)
    out = np.empty((B * H, N, D), dtype=np.float32)
    for c in range(NCORES):
        out[HPC * c : HPC * (c + 1)] = res.results[c]["out"]
    return out.reshape(B, H, N, D)


# revision 21
# speedup vs baseline: 1.2285x; 1.0094x over previous
"""DCT-compressed attention (nn_DCTAttentionIdeal) on 8 Trainium2 NeuronCores.

Math (per head, reference ordering):
    S    = (Q*s) @ (K*mask*s)^T with s = D**-0.25             [N,N]
    atn  = softmax(S, axis=-1)
    Vd   = Qd @ (V*mask)                                      [M,D]
    out  = Qd^T @ ((Qd @ atn @ Qd^T) @ Vd)                    [N,D]

Kernel reassociation (exact in real arithmetic):
    out = Qd^T @ (Qd @ (atn @ U)),   U = Qd^T @ (Qd @ (V*mask))  [N,D]
so the only O(N^2) contractions are the scores (N^2 D) and atn @ U
(N^2 D) -- the N^2 M path of the naive ordering is gone.

Implementation notes:
  - scores are computed TRANSPOSED (S^T[k,q]) so that exp(S^T) tiles act
    directly as matmul weights for the k-contraction of atn @ U, and the
    softmax denominator comes free as a ones-column appended to U.
  - exp is stored bf16 (fp8 mantissa is too coarse: exp quantization
    error passes straight through atn@U since Y is a random-sign sum).
  - S^T uses 2x PE row tiling (contraction D=64): even k-blocks on array
    rows 0-63, odd k-blocks on rows 64-127, chunk matmuls interleaved
    across row groups so every LDWEIGHTS hides under the other group's
    stream.
  - The kernel is paced by ScalarE's exp stream (128 activations of
    [128,1024]).  Emission is act-paced: each pair-half slot emits 4
    score matmuls + 2 activations, then pops two "filler atoms" (~8-16
    matmul slices of atn@U / DCT-tail / U-prep work for neighboring
    heads), so the strict-FIFO TensorE queue never idles on the
    act-region rotation with runnable work stuck behind it.
  - atn@U accumulators alternate between two PSUM banks so the DVE
    normalize of block q never blocks the matmuls of block q+1.
  - heads are processed in pairs for the small DCT matmuls (Vd/U/Z1/out):
    two heads' 64-wide streams share one 128-wide weight load.
  - PSUM: 6 banks = 3-deep rotation of [128,1024] score regions, 2 banks
    for the alternating atn@U accumulator / misc tail.

Sharding: batch*heads (2*16=32) split 4-per-core across 8 cores; Q_dct
replicated; no cross-core communication.  Host does layout only
(transposes / duplication for row tiling / mask broadcast layouts);
masking, softmax and all DCT algebra run on-device.
"""

from collections import deque

import numpy as np
import ml_dtypes

import concourse.tile as tile
from concourse import bacc, mybir
from concourse import bass_utils

F32 = mybir.dt.float32
BF16 = mybir.dt.bfloat16
I16 = mybir.dt.int16
ALU = mybir.AluOpType
NPBF16 = ml_dtypes.bfloat16
AF = mybir.ActivationFunctionType

B, H, N, D, M = 2, 16, 2048, 64, 256
NCORES = 8
HPC = (B * H) // NCORES  # heads per core = 4
P = 128
NT = N // P              # 16 k/q blocks
MT = M // P              # 2
SCH_A = 0.125 * 128.0 / np.log(2.0)   # Schraudolph exp: bf16 bits = A*s + B
SCH_B = 128.0 * 127.0 - 7.5


def _emit(tc, ctx, io):
    nc = tc.nc

    sh = ctx.enter_context(tc.tile_pool(name="shared", bufs=1))
    in2 = ctx.enter_context(tc.tile_pool(name="inputs", bufs=3))
    v2_pool = ctx.enter_context(tc.tile_pool(name="vpair", bufs=2))
    exp_pool = ctx.enter_context(tc.tile_pool(name="exp", bufs=2))
    ua_pool = ctx.enter_context(tc.tile_pool(name="ua", bufs=2))
    vd_pool = ctx.enter_context(tc.tile_pool(name="vd", bufs=2))
    au_pool = ctx.enter_context(tc.tile_pool(name="atnun", bufs=2))
    z1_pool = ctx.enter_context(tc.tile_pool(name="z1", bufs=2))
    ost_pool = ctx.enter_context(tc.tile_pool(name="ost", bufs=1))
    st_pool = ctx.enter_context(tc.tile_pool(name="stats", bufs=4))

    ps_st = ctx.enter_context(tc.tile_pool(name="ps_st", bufs=3, space="PSUM"))
    ps_a = ctx.enter_context(tc.tile_pool(name="ps_a", bufs=1, space="PSUM"))
    ps_m = ctx.enter_context(tc.tile_pool(name="ps_m", bufs=1, space="PSUM"))
    _ctr = [0]

    def next_ps(width):
        # alternate the two spare PSUM banks between consecutive atoms so an
        # atom's matmuls never wait for the previous atom's DVE evacuation
        pool, tag = (ps_a, "psa") if _ctr[0] % 2 == 0 else (ps_m, "misc")
        _ctr[0] += 1
        return pool.tile([P, width], F32, name="ps", tag=tag)

    # --- shared, once per core (mask layouts first: first act needs them)
    mkb = sh.tile([P, NT // 2, P], BF16)  # key mask in KT2 layout
    nc.sync.dma_start(mkb[:], io["maskKB"])

    state = [None] * HPC

    def prep_dma(h):
        st = state[h] = {}
        qk = in2.tile([P, N + (NT // 2) * P], BF16, name="qk", tag="qk")
        nc.sync.dma_start(qk[:], io["QK2"][h])
        st["qt2"] = qk[:, 0:N]
        st["kt2"] = qk[:, N:].rearrange("p (a b) -> p a b", b=P)
        if h % 2 == 0:
            st["v2"] = v2_pool.tile([P, NT, 2, D], BF16, name="v2", tag="v2")
        else:
            st["v2"] = state[h - 1]["v2"]
        nc.sync.dma_start(
            st["v2"][:, :, h % 2, :], io["V"][h].rearrange("(t p) d -> p t d", p=P)
        )

    prep_dma(0)
    mvb = sh.tile([P, NT, D], BF16)       # value mask in V layout
    nc.sync.dma_start(mvb[:], io["maskVB"])

    qdtr = sh.tile([P, NT, M], BF16)    # Qd^T[n, m] -> [p, t, m]
    nc.sync.dma_start(qdtr[:], io["QdTr"].rearrange("(t p) m -> p t m", p=P))
    qdnr = sh.tile([P, MT, N], BF16)    # Qd[m, q]   -> [p, c, q]
    nc.sync.dma_start(qdnr[:], io["QdNr"].rearrange("(c p) q -> p c q", p=P))

    if HPC > 1:
        prep_dma(1)

    # dummy activation: pull the exp table load off the critical path
    scr = sh.tile([P, 1], F32)
    nc.vector.memset(scr[:], 0.0)
    nc.scalar.activation(scr[:], scr[:], AF.Exp)

    mkbf = mkb.rearrange("p a b -> p (a b)")

    def dummy_mms(n):
        """Keep the PE's HAM activity monitor busy (K=8/8) when the pipeline
        has no real TensorE work: harmless matmuls over the mask tile."""
        psd = next_ps(512)
        for _ in range(n):
            nc.tensor.matmul(
                psd[:], lhsT=mkbf[:, 0:P], rhs=mkbf[:, 0:512], start=True, stop=True
            )

    # warm-up burst: ~7us of back-to-back matmuls during the DMA wait flips
    # the HAM clock gate to full speed before the real pipeline starts.
    dummy_mms(16)

    def mask_inputs(h):
        st = state[h]
        nc.vector.tensor_mul(st["kt2"][:], st["kt2"][:], mkb[:])
        nc.vector.tensor_mul(
            st["v2"][:, :, h % 2, :], st["v2"][:, :, h % 2, :], mvb[:]
        )

    # ---- filler atoms (TensorE work hidden inside the act-paced spine) --
    # Small DCT matmuls run head-PAIRED: rhs [128, 2*64] spans both heads,
    # so one 128-column weight load feeds 128 streamed columns.
    def vd_atom(hp):
        """Vd = Qd @ (V*m) for head pair hp -> vd2 [128, 2, 2, 64]."""
        st = state[2 * hp]
        vd2 = st["vd2"] = vd_pool.tile([P, MT, 2, D], BF16, name="vd2", tag="vd2")
        psv = next_ps(512)
        for mh in range(MT):
            for t in range(NT):
                nc.tensor.matmul(
                    psv[:, mh * 2 * D : (mh + 1) * 2 * D],
                    lhsT=qdtr[:, t, mh * P : (mh + 1) * P],
                    rhs=st["v2"][:, t, :, :],
                    start=(t == 0),
                    stop=(t == NT - 1),
                )
        nc.vector.tensor_copy(
            vd2[:], psv[:, 0 : MT * 2 * D].rearrange("p (c h d) -> p c h d", h=2, d=D)
        )

    def u_atom(hp, quarter):
        """U = Qd^T @ Vd for 4 k-blocks of head pair hp (+ ones column)."""
        st = state[2 * hp]
        if quarter == 0:
            st["ua2"] = ua_pool.tile([P, NT, 2, D + 1], BF16, name="ua2", tag="ua2")
        ua2 = st["ua2"]
        psu = next_ps(512)
        for k4 in range(4):
            kc = quarter * 4 + k4
            for mh in range(MT):
                nc.tensor.matmul(
                    psu[:, k4 * 2 * D : (k4 + 1) * 2 * D],
                    lhsT=qdnr[:, mh, kc * P : (kc + 1) * P],
                    rhs=st["vd2"][:, mh, :, :],
                    start=(mh == 0),
                    stop=(mh == MT - 1),
                )
        nc.vector.tensor_copy(
            ua2[:, quarter * 4 : (quarter + 1) * 4, :, 0:D],
            psu[:].rearrange("p (c h d) -> p c h d", h=2, d=D),
        )
        nc.vector.memset(ua2[:, quarter * 4 : (quarter + 1) * 4, :, D : D + 1], 1.0)

    def atnu_atom(h, q):
        """One q-block of Y = atn @ [U|1]: 16 matmuls + normalize.
        Accumulator bank alternates between ps_a and ps_m so the DVE
        normalize of block q never blocks the matmuls of block q+1."""
        st = state[h]
        hp2 = h % 2
        if q == 0 and hp2 == 0:
            state[h]["au2"] = au_pool.tile([P, NT, 2, D], BF16, name="au2", tag="au2")
        if q == 0 and hp2 == 1:
            state[h]["au2"] = state[h - 1]["au2"]
        au2 = st["au2"]
        ua2 = state[h - hp2]["ua2"]
        psa = next_ps(D + 1)
        for kc in range(NT):
            nc.tensor.matmul(
                psa[:],
                lhsT=st["ex"][:, kc, q * P : (q + 1) * P],
                rhs=ua2[:, kc, hp2, :],
                start=(kc == 0),
                stop=(kc == NT - 1),
            )
        rec = st_pool.tile([P, 1], F32, name="rec", tag="rec")
        nc.vector.reciprocal(rec[:], psa[:, D : D + 1])
        if h == HPC - 1:
            nc.scalar.mul(au2[:, q, hp2, :], psa[:, 0:D], rec[:])
        else:
            nc.vector.tensor_scalar_mul(au2[:, q, hp2, :], psa[:, 0:D], rec[:])

    def z1_atom(hp):
        """Z1 = Qd @ Y for head pair hp -> z12 [128, 2, 2, 64]."""
        st = state[2 * hp]
        z12 = st["z12"] = z1_pool.tile([P, MT, 2, D], BF16, name="z12", tag="z12")
        psz = next_ps(512)
        for mh in range(MT):
            for q in range(NT):
                nc.tensor.matmul(
                    psz[:, mh * 2 * D : (mh + 1) * 2 * D],
                    lhsT=qdtr[:, q, mh * P : (mh + 1) * P],
                    rhs=st["au2"][:, q, :, :],
                    start=(q == 0),
                    stop=(q == NT - 1),
                )
        nc.vector.tensor_copy(
            z12[:], psz[:, 0 : MT * 2 * D].rearrange("p (c h d) -> p c h d", h=2, d=D)
        )

    def out_atom(hp, quarter):
        """out = Qd^T @ Z1 for 4 q-blocks of head pair hp; DMA at the end."""
        st = state[2 * hp]
        if quarter == 0:
            st["ost2"] = ost_pool.tile([P, NT, 2, D], F32, name="ost2", tag="ost2")
        ost2 = st["ost2"]
        pso = next_ps(512)
        for q4 in range(4):
            q = quarter * 4 + q4
            for mh in range(MT):
                nc.tensor.matmul(
                    pso[:, q4 * 2 * D : (q4 + 1) * 2 * D],
                    lhsT=qdnr[:, mh, q * P : (q + 1) * P],
                    rhs=st["z12"][:, mh, :, :],
                    start=(mh == 0),
                    stop=(mh == MT - 1),
                )
        nc.vector.tensor_copy(
            ost2[:, quarter * 4 : (quarter + 1) * 4, :, :],
            pso[:].rearrange("p (c h d) -> p c h d", h=2, d=D),
        )
        if quarter % 2 == 1:
            for hh in range(2):
                nc.sync.dma_start(
                    io["out"][2 * hp + hh]
                    .rearrange("(t p) d -> p t d", p=P)[:, (quarter - 1) * 4 : (quarter + 1) * 4, :],
                    ost2[:, (quarter - 1) * 4 : (quarter + 1) * 4, hh, :],
                )
        if quarter == 3:
            state[2 * hp] = state[2 * hp + 1] = None

    def phase_atoms(h):
        """Fillers for the S-phase of head h (indices may refer to earlier
        heads' work whose inputs completed in previous phases).  Dummy
        matmuls pad phases whose real TensorE work is far below the act
        spine, so the HAM clock gate never re-throttles the PE."""
        atoms = []
        prep = []
        if h % 2 == 1 and h < HPC:
            hp = h // 2  # prep for the pair whose 2nd head phase this is
            prep.append(lambda hp=hp: vd_atom(hp))
            for qu in range(4):
                prep.append(lambda hp=hp, qu=qu: u_atom(hp, qu))
        work = []
        if 0 <= h - 1 < HPC:
            for q in range(NT):
                work.append(lambda h=h, q=q: atnu_atom(h - 1, q))
        tail = []
        if h - 2 >= 1 and (h - 2) % 2 == 1:
            hp = (h - 2) // 2  # tail for the pair completed at end of h-1
            tail.append(lambda hp=hp: z1_atom(hp))
            for qu in range(4):
                tail.append(lambda hp=hp, qu=qu: out_atom(hp, qu))
        n_dum = {0: 10, 1: 2, 2: 2, 3: 2}.get(h, 0)
        atoms = prep + work + tail
        # interleave dummies after every 2nd real atom
        if n_dum:
            out = []
            k = 0
            for a in atoms:
                out.append(a)
                k += 1
                if k % 2 == 0 and n_dum > 0:
                    out.append(lambda: dummy_mms(4))
                    n_dum -= 1
            out.extend([lambda: dummy_mms(4)] * n_dum)
            atoms = out
        return atoms

    def s_pair_half(h, j, half):
        """Score k-blocks (2j, 2j+1) for one q-half: 4 matmuls interleaved
        across PE row groups + 2 exp evaluations.  In phases 0 and HPC-1
        (which lack filler work / gate the epilogue) some units compute exp
        on the DVE instead via the Schraudolph bit trick: bf16 bits =
        round(A*scores + B), evaluated as one tensor_scalar into an int16
        view of the exp tile (rel err ~1.8%, cancels partly in softmax)."""
        st = state[h]
        slot = 2 * j + half
        if h == 0:
            sch = 2 <= slot <= 13
        elif h == HPC - 1:
            sch = slot <= 11
        else:
            sch = 8 <= slot <= 13
        pss = []
        for side in range(2):
            pss.append(ps_st.tile([P, 1024], F32, name="st", tag="st"))
        for c in range(2):
            for side in range(2):
                pr = slice(64 * side, 64 * (side + 1))
                q0 = half * 1024 + c * 512
                nc.tensor.matmul(
                    pss[side][:, c * 512 : (c + 1) * 512],
                    lhsT=st["kt2"][pr, j, :],
                    rhs=st["qt2"][pr, q0 : q0 + 512],
                    start=True,
                    stop=True,
                )
        for side in range(2):
            kc = 2 * j + side
            dst = st["ex"][:, kc, half * 1024 : (half + 1) * 1024]
            if sch and side == 1:
                nc.vector.tensor_scalar(
                    dst.bitcast(I16), pss[side][:], SCH_A, SCH_B,
                    op0=ALU.mult, op1=ALU.add,
                )
            else:
                nc.scalar.activation(dst, pss[side][:], AF.Exp, scale=0.125)

    # --- act-paced pipelined emission -----------------------------------
    mask_inputs(0)
    atoms = deque()
    for h in range(HPC):
        st = state[h]
        st["ex"] = exp_pool.tile([P, NT, N], BF16, name="ex", tag="ex")
        atoms.extend(phase_atoms(h))
        for slot in range(NT):
            j, half = divmod(slot, 2)
            if slot == 4 and h + 1 < HPC:
                mask_inputs(h + 1)
            if slot == 8 and h + 2 < HPC:
                prep_dma(h + 2)
            s_pair_half(h, j, half)
            if h > 0 or slot >= 2:
                want = (len(atoms) + (NT - 1 - slot)) // (NT - slot)
                for _ in range(min(want, 3)):
                    if atoms:
                        atoms.popleft()()
    for h in (HPC, HPC + 1):
        atoms.extend(phase_atoms(h))
    while atoms:
        atoms.popleft()()


def build_nc():
    from contextlib import ExitStack

    nc = bacc.Bacc("TRN2", target_bir_lowering=False, debug=False)
    io = {
        "QK2": nc.dram_tensor("QK2", [HPC, P, N + (NT // 2) * P], BF16, kind="ExternalInput").ap(),
        "V": nc.dram_tensor("V", [HPC, N, D], BF16, kind="ExternalInput").ap(),
        "maskKB": nc.dram_tensor("maskKB", [P, NT // 2, P], BF16, kind="ExternalInput").ap(),
        "maskVB": nc.dram_tensor("maskVB", [P, NT, D], BF16, kind="ExternalInput").ap(),
        "QdTr": nc.dram_tensor("QdTr", [N, M], BF16, kind="ExternalInput").ap(),
        "QdNr": nc.dram_tensor("QdNr", [M, N], BF16, kind="ExternalInput").ap(),
        "out": nc.dram_tensor("out", [HPC, N, D], F32, kind="ExternalOutput").ap(),
    }
    with tile.TileContext(nc) as tc:
        with ExitStack() as ctx:
            _emit(tc, ctx, io)
    nc.compile()
    return nc


_NC = None


def _get_nc():
    global _NC
    if _NC is None:
        _NC = build_nc()
    return _NC


def make_in_maps(Q, K, V, mask, Q_dct):
    Q = np.asarray(Q, dtype=np.float32).reshape(B * H, N, D)
    K = np.asarray(K, dtype=np.float32).reshape(B * H, N, D)
    V = np.asarray(V, dtype=np.float32).reshape(B * H, N, D)
    mask = np.asarray(mask, dtype=np.float32)
    Q_dct = np.asarray(Q_dct, dtype=np.float32)

    QT = np.ascontiguousarray(Q.transpose(0, 2, 1))          # [BH, 64, N]
    KT = np.ascontiguousarray(K.transpose(0, 2, 1))          # [BH, 64, N]
    # duplicate Q^T across both partition halves (PE row tiling)
    QT2 = np.concatenate([QT, QT], axis=1)                   # [BH, 128, N]
    # interleave K^T k-blocks: even blocks on partitions 0-63, odd on 64-127
    KTb = KT.reshape(B * H, D, NT, P)                        # [BH, 64, 16, 128]
    KT2 = np.concatenate([KTb[:, :, 0::2, :], KTb[:, :, 1::2, :]], axis=1)
    QdTr = np.ascontiguousarray(Q_dct.T).astype(NPBF16)
    QdNr = np.ascontiguousarray(Q_dct).astype(NPBF16)

    in_maps = []
    for c in range(NCORES):
        sl = slice(HPC * c, HPC * (c + 1))
        heads = list(range(HPC * c, HPC * (c + 1)))
        bs = {hp // H for hp in heads}
        assert len(bs) == 1, "all heads on a core must share a batch row"
        b = bs.pop()
        # key mask in KT2 layout [128, 8, 128] (broadcast over d-partitions)
        mk = mask[b].reshape(NT, P)                          # [16 blocks, 128]
        mkb = np.empty((P, NT // 2, P), dtype=np.float32)
        mkb[0:64] = mk[0::2][None, :, :]
        mkb[64:128] = mk[1::2][None, :, :]
        # value mask in V-tile layout [128, 16, 64]
        mvb = np.broadcast_to(mk.T[:, :, None], (P, NT, D))
        qk2 = np.concatenate(
            [QT2[sl], KT2[sl].reshape(HPC, P, (NT // 2) * P)], axis=2
        )
        in_maps.append(
            {
                "QK2": np.ascontiguousarray(qk2).astype(NPBF16),
                "V": np.ascontiguousarray(V[sl]).astype(NPBF16),
                "maskKB": np.ascontiguousarray(mkb).astype(NPBF16),
                "maskVB": np.ascontiguousarray(mvb).astype(NPBF16),
                "QdTr": QdTr,
                "QdNr": QdNr,
            }
        )
    return in_maps


def run_on_device(in_maps, **kwargs):
    nc = _get_nc()
    return bass_utils.run_bass_kernel_spmd(
        nc, in_maps, core_ids=list(range(NCORES)), **kwargs
    )


def kernel(Q, K, V, mask, Q_dct):
    in_maps = make_in_maps(Q, K, V, mask, Q_dct)
    res = run_on_device(in_maps)
    out = np.empty((B * H, N, D), dtype=np.float32)
    for c in range(NCORES):
        out[HPC * c : HPC * (c + 1)] = res.results[c]["out"]
    return out.reshape(B, H, N, D)
